# revision 1
# baseline (speedup 1.0000x reference)
"""Trainium2 Bass kernel for nn_Connector_77738908057780 (dense_mlp).

Computation (see reference):
  x   = image_features                      [B, N, H]    bf16
  f1  = mean(hidden[0:13],  axis=0)         [B, N, H]
  f2  = mean(hidden[13:26], axis=0)         [B, N, H]
  cat = concat([x, f1, f2], -1)             [B, N, 3H]
  h   = gelu(cat @ W1.T + b1)               W1 = nf4_dequant(codes1, scales1) [H, 3H]
  fg  = h @ W2.T + b2                       W2 = nf4_dequant(codes2, scales2) [H, H]
  out = w * LN(fg) + (1-w) * LN(x),         w = sigmoid(alpha)

Sharding: data-parallel over batch B=8 -> one batch element per NeuronCore.

v2 design (vs the 286us baseline):
  - skewed software pipeline: supertile st's LN/gate/store stage is emitted
    during st+1 so the DVE never idles waiting on GEMM2.
  - hidden streamed as 13 layer-PAIR DMAs per supertile (1.18 MB each), all
    on the sync HWDGE queue; transposes/weights/consts ride the scalar queue
    so neither blocks the other (separate FIFO rows, SDMA round-robins).
  - the 26-layer sums are split DVE (fast chains) + GpSimd (leading pair
    folds) so DVE load drops from ~155us to ~110us and the DMA never stalls
    on a consumer.
  - GEMM1 is k-eager: all 9 m-tiles accumulate in 4.5 PSUM banks and the
    per-k matmuls fire as soon as each cat^T k-group lands (x first, then
    f1, then f2) - keeps the PE warm (HAM throttle) and off the tail.
  - b2 is folded into GEMM2 as a 10th k-tile (all-ones stationary column x
    a [b2; 0...] row block), killing the DVE bias adds.
  - NF4 dequant of the (small, replicated) weights is host-side weight prep.
"""

import os
import sys

import numpy as np
import ml_dtypes

for _p in ("/opt/trn_rl_repo", "/root/.axon_site/_ro/trn_rl_repo"):
    if os.path.isdir(_p) and _p not in sys.path:
        sys.path.insert(0, _p)

import concourse.bass as bass
import concourse.mybir as mybir
import concourse.tile as tile
from concourse import bacc
from concourse import bass_utils

BF16 = mybir.dt.bfloat16
F32 = mybir.dt.float32
AF = mybir.ActivationFunctionType
ALU = mybir.AluOpType

NP_BF16 = ml_dtypes.bfloat16

P = 128
H = 1152
H3 = 3456
NT = 729          # tokens per core (N); B=8 cores
L = 26
KO1 = H3 // P     # 27 k-tiles for GEMM1
KO2 = H // P      # 9 k-tiles for GEMM2 (+1 ones-tile for the b2 fold)
MO = H // P       # 9 output-feature tiles
EPS = 1e-5
NCHUNK = 3        # fg free-dim chunks of 384
CH = H // NCHUNK  # 384

# Supertiles of exactly 256 tokens; the last overlaps the previous by 39
# tokens (473..511 computed twice, identical values stored twice) so that
# every DMA/compute tile is a full 128-partition tile (729 is not a
# multiple of 128; partial-partition tiles hit HW-hostile DMA paths).
SUPERTILES = [0, 256, 473]
TSUP = 256        # tokens per supertile
NSUB = 2          # 128-token subtiles per supertile

NF4_CODEBOOK = np.array([
    -1.0, -0.6961928009986877, -0.5250730514526367, -0.39491748809814453,
    -0.28444138169288635, -0.18477343022823334, -0.09105003625154495, 0.0,
    0.07958029955625534, 0.16093020141124725, 0.24611230194568634,
    0.33791524171829224, 0.4407098591327667, 0.5626170039176941,
    0.7229568362236023, 1.0], dtype=np.float32)

BLOCK = 64


def _dequant_nf4(codes, scales):
    """Match reference: codebook lookup * per-64-block absmax, cast bf16."""
    out_f, in_f = codes.shape
    w = NF4_CODEBOOK[codes].reshape(out_f, in_f // BLOCK, BLOCK)
    w = w * scales[:, :, None].astype(np.float32)
    return w.reshape(out_f, in_f)  # float32 (caller casts)


def _build_program(act=AF.Gelu):
    nc = bacc.Bacc(
        "TRN2",
        target_bir_lowering=False,
        debug=False,
        num_devices=1,
    )
    x_d = nc.dram_tensor("x", (NT, H), BF16, kind="ExternalInput").ap()
    hid_d = nc.dram_tensor("hid", (L, NT, H), BF16, kind="ExternalInput").ap()
    w1t_d = nc.dram_tensor("w1t", (H3, H), BF16, kind="ExternalInput").ap()
    w2t_d = nc.dram_tensor("w2t", ((KO2 + 1) * P, H), BF16,
                           kind="ExternalInput").ap()
    b1s_d = nc.dram_tensor("b1s", (P, MO), F32, kind="ExternalInput").ap()
    g1b_d = nc.dram_tensor("g1b", (P, H), BF16, kind="ExternalInput").ap()
    g2b_d = nc.dram_tensor("g2b", (P, H), BF16, kind="ExternalInput").ap()
    bcb_d = nc.dram_tensor("bcb", (P, H), BF16, kind="ExternalInput").ap()
    id_d = nc.dram_tensor("ident", (P, P), BF16, kind="ExternalInput").ap()
    out_d = nc.dram_tensor("out", (NT, H), BF16, kind="ExternalOutput").ap()

    with tile.TileContext(nc) as tc:
        _program(nc, tc, x_d, hid_d, w1t_d, w2t_d, b1s_d,
                 g1b_d, g2b_d, bcb_d, id_d, out_d, act)

    nc.compile()
    return nc


def _program(nc, tc, x_d, hid_d, w1t_d, w2t_d, b1s_d, g1b_d, g2b_d,
             bcb_d, id_d, out_d, act=AF.Gelu):
    with (
        tc.tile_pool(name="consts", bufs=1) as cpool,
        tc.tile_pool(name="hp", bufs=3) as hpool,
        tc.tile_pool(name="hpp", bufs=2) as hppool,
        tc.tile_pool(name="acc", bufs=2) as apool,
        tc.tile_pool(name="catx", bufs=2) as cxpool,
        tc.tile_pool(name="catf", bufs=1) as cfpool,
        tc.tile_pool(name="gt", bufs=1) as gpool,
        tc.tile_pool(name="xn", bufs=2) as xpool,
        tc.tile_pool(name="fg", bufs=2) as fgpool,
        tc.tile_pool(name="outp", bufs=2) as opool,
        tc.tile_pool(name="stats", bufs=2) as spool,
        tc.tile_pool(name="tmp", bufs=1) as tpool,
        tc.tile_pool(name="psA", bufs=5, space="PSUM") as psapool,
        tc.tile_pool(name="ps2", bufs=1, space="PSUM") as ps2pool,
        tc.tile_pool(name="psT", bufs=2, space="PSUM") as pstpool,
    ):
        # ---- constants ----
        ones_sb = cpool.tile([P, TSUP], BF16)
        nc.vector.memset(ones_sb[:, :], 1.0)
        id_sb = cpool.tile([P, P], BF16)
        nc.scalar.dma_start(id_sb, id_d)
        b1_sb = cpool.tile([P, MO], F32)
        g1b_sb = cpool.tile([P, H], BF16)
        g2b_sb = cpool.tile([P, H], BF16)
        bcb_sb = cpool.tile([P, H], BF16)
        w1t_sb = cpool.tile([P, KO1, H], BF16)
        w2t_sb = cpool.tile([P, KO2 + 1, H], BF16)
        nc.scalar.dma_start(b1_sb, b1s_d)
        nc.scalar.dma_start(g1b_sb, g1b_d)
        nc.scalar.dma_start(g2b_sb, g2b_d)
        nc.scalar.dma_start(bcb_sb, bcb_d)

        w1t_r = w1t_d.rearrange("(ko p) n -> p ko n", p=P)
        w2t_r = w2t_d.rearrange("(ko p) n -> p ko n", p=P)

        def pe_transpose(dst, src):
            """128x128 transpose on the TensorEngine (identity matmul,
            bf16 PSUM out, one tile per 2KB bank), copied out by ACT.
            No DMA involvement: xbar DMA-transposes serialize against
            in-flight DMA traffic and stall the pipe. GpSimd is NOT used
            for any of this: concurrent Pool-engine tensor ops slow DVE
            2x-mode ops ~4x (SBUF port contention)."""
            psT = pstpool.tile([P, P], BF16, tag="psT", name="psT")
            nc.tensor.transpose(psT, src, id_sb)
            nc.scalar.activation(dst, psT, AF.Copy)

        # Skewed tail for supertile `pv` (ran GEMM2 late in the NEXT
        # supertile's window): emitted piecewise inside pass st+1.
        def tail_stats(pv):
            """LN2 stats + rsqrt input for the previous supertile (DVE),
            followed by the rsqrt chain start."""
            (p_x, p_fgs, p_agg, p_rpack, p_t0) = pv
            for tt in range(NSUB):
                bnf = spool.tile([P, 3, 6], F32, tag="bnf")
                for c in range(NCHUNK):
                    nc.vector.bn_stats(bnf[:, c, :],
                                       p_fgs[tt][:, c * CH:(c + 1) * CH])
                nc.vector.bn_aggr(p_agg[:, tt, 2:4], bnf)
                nc.vector.tensor_scalar_add(
                    p_rpack[:, 2 * tt + 1:2 * tt + 2],
                    p_agg[:, tt, 3:4], EPS)
            ig = spool.tile([P, 2 * NSUB], F32, tag="ig")
            nc.vector.reciprocal(ig, p_rpack)
            bv = spool.tile([P, 2 * NSUB], F32, tag="bv")
            return ig, bv

        def tail_norm(pv, ig, bv):
            """ACT: sqrt + the per-token normalizations
            xn = (v - mu) * rsqrt as Identity activations (scale=rsqrt,
            bias=-mu*rsqrt). Identity/Copy live in every act table."""
            (p_x, p_fgs, p_agg, p_rpack, p_t0) = pv
            nc.scalar.activation(ig, ig, AF.Sqrt)
            xns = []
            for tt in range(NSUB):
                for ln in range(2):
                    s = 2 * tt + ln
                    nc.vector.tensor_scalar(
                        bv[:, s:s + 1], p_agg[:, tt, 2 * ln:2 * ln + 1],
                        ig[:, s:s + 1], -1.0, ALU.mult, ALU.mult)
                xn1 = tpool.tile([P, H], BF16, tag="xn1")
                nc.scalar.activation(xn1, p_x[:, tt, :], AF.Identity,
                                     bias=bv[:, 2 * tt:2 * tt + 1],
                                     scale=ig[:, 2 * tt:2 * tt + 1])
                xn2 = tpool.tile([P, H], BF16, tag="xn2")
                nc.scalar.activation(xn2, p_fgs[tt], AF.Identity,
                                     bias=bv[:, 2 * tt + 1:2 * tt + 2],
                                     scale=ig[:, 2 * tt + 1:2 * tt + 2])
                xns.append((xn1, xn2))
            return xns

        def tail_combine(xns):
            """DVE, all-bf16 2x-mode TTs: out = xn1*G1 + xn2*G2 + Bc."""
            outs = []
            for tt in range(NSUB):
                xn1, xn2 = xns[tt]
                nc.vector.tensor_tensor(xn1, xn1, g1b_sb, ALU.mult)
                nc.vector.tensor_tensor(xn2, xn2, g2b_sb, ALU.mult)
                nc.vector.tensor_tensor(xn1, xn1, xn2, ALU.add)
                out_t = opool.tile([P, H], BF16, tag="outt")
                nc.vector.tensor_tensor(out_t, xn1, bcb_sb, ALU.add)
                outs.append(out_t)
            return outs

        prev = None

        def emit_gemm2_tt(gTp, tt):
            """GEMM2 for one 128-token subtile of the PREVIOUS supertile
            (+b2 via all-ones stationary x [b2;0..] k-tile), PSUM drained
            by ACT copies."""
            fg = fgpool.tile([P, H], BF16, tag="fg", name=f"fg{tt}")
            for nn in range(NCHUNK):
                ps2 = ps2pool.tile([P, 512], F32, tag="ps2", name="ps2")
                for kk in range(KO2):
                    nc.tensor.matmul(
                        ps2[:, 0:CH],
                        lhsT=gTp[:, kk, tt * P:(tt + 1) * P],
                        rhs=w2t_sb[:, kk, nn * CH:(nn + 1) * CH],
                        start=(kk == 0),
                        stop=False,
                    )
                nc.tensor.matmul(
                    ps2[:, 0:CH],
                    lhsT=ones_sb[:, tt * P:(tt + 1) * P],
                    rhs=w2t_sb[:, KO2, nn * CH:(nn + 1) * CH],
                    start=False,
                    stop=True,
                )
                nc.scalar.activation(fg[:, nn * CH:(nn + 1) * CH],
                                     ps2[:, 0:CH], AF.Copy)
            return fg

        def emit_bnf_tt(pv, fg, tt):
            (p_x, p_gT, p_agg, p_rpack, p_t0) = pv
            bnf = spool.tile([P, 3, 6], F32, tag="bnf")
            for c in range(NCHUNK):
                nc.vector.bn_stats(bnf[:, c, :], fg[:, c * CH:(c + 1) * CH])
            nc.vector.bn_aggr(p_agg[:, tt, 2:4], bnf)
            nc.vector.tensor_scalar_add(p_rpack[:, 2 * tt + 1:2 * tt + 2],
                                        p_agg[:, tt, 3:4], EPS)

        def emit_tail_norm(pv, fgs_p):
            """rsqrt + per-token normalizations xn = (v - mu) * rsqrt as
            Identity activations (scale=rsqrt, bias=-mu*rsqrt; both [P,1]
            APs). Identity/Copy live in every act table."""
            (p_x, p_gT, p_agg, p_rpack, p_t0) = pv
            ig = spool.tile([P, 2 * NSUB], F32, tag="ig")
            nc.vector.reciprocal(ig, p_rpack)
            nc.scalar.activation(ig, ig, AF.Sqrt)
            bv = spool.tile([P, 2 * NSUB], F32, tag="bv")
            xns = []
            for tt in range(NSUB):
                for ln in range(2):
                    s = 2 * tt + ln
                    nc.vector.tensor_scalar(
                        bv[:, s:s + 1], p_agg[:, tt, 2 * ln:2 * ln + 1],
                        ig[:, s:s + 1], -1.0, ALU.mult, ALU.mult)
                xn1 = tpool.tile([P, H], BF16, tag="xn1")
                nc.scalar.activation(xn1, p_x[:, tt, :], AF.Identity,
                                     bias=bv[:, 2 * tt:2 * tt + 1],
                                     scale=ig[:, 2 * tt:2 * tt + 1])
                xn2 = tpool.tile([P, H], BF16, tag="xn2")
                nc.scalar.activation(xn2, fgs_p[tt], AF.Identity,
                                     bias=bv[:, 2 * tt + 1:2 * tt + 2],
                                     scale=ig[:, 2 * tt + 1:2 * tt + 2])
                xns.append((xn1, xn2))
            return xns

        def emit_tail_combine(pv, xns):
            """DVE all-bf16 2x TTs: out = xn1*G1 + xn2*G2 + Bc, then store."""
            for tt in range(NSUB):
                xn1, xn2 = xns[tt]
                nc.vector.tensor_tensor(xn1, xn1, g1b_sb, ALU.mult)
                nc.vector.tensor_tensor(xn2, xn2, g2b_sb, ALU.mult)
                nc.vector.tensor_tensor(xn1, xn1, xn2, ALU.add)
                out_t = opool.tile([P, H], BF16, tag="outt")
                nc.vector.tensor_tensor(out_t, xn1, bcb_sb, ALU.add)
                nc.sync.dma_start(
                    out_d[pv[4] + tt * P:pv[4] + (tt + 1) * P, :], out_t)

        for st_idx, t0 in enumerate(SUPERTILES):
            # ---- ALL loads on the SP queue; hidden as 6-layer-block DMAs
            # per 128-token subtile (1.77 MB each, 3-dim APs) + one l12/l13
            # pair: 11 loads per supertile, so the single SP sequencer
            # never rate-limits the DMA engines. ----
            x_nat = xpool.tile([P, NSUB, H], BF16, tag="xnat")
            nc.sync.dma_start(
                x_nat,
                x_d[t0:t0 + TSUP, :].rearrange("(s p) f -> p s f", p=P),
            )

            def load_block(l0, nl, tt, name):
                pool = hpool if nl == 6 else hppool
                bt = pool.tile([P, nl, H], BF16, name=name,
                               tag="hp" if nl == 6 else "hpp")
                nc.sync.dma_start(
                    bt,
                    hid_d[l0:l0 + nl,
                          t0 + tt * P:t0 + (tt + 1) * P, :].rearrange(
                        "l p f -> p l f"))
                return bt

            blk = {}
            wsched = {0: 0, 2: 9, 5: 18, 7: None} if st_idx == 0 else {}
            bi = 0
            for l0, nl in ((0, 6), (6, 6), (12, 2), (14, 6), (20, 6)):
                for tt in range(NSUB):
                    blk[(l0, tt)] = load_block(l0, nl, tt, f"b{l0}t{tt}")
                    if bi in wsched:
                        c0 = wsched[bi]
                        if c0 is not None:
                            nc.sync.dma_start(w1t_sb[:, c0:c0 + 9, :],
                                              w1t_r[:, c0:c0 + 9, :])
                        else:
                            nc.sync.dma_start(w2t_sb, w2t_r)
                    bi += 1

            # ---- x^T tiles (PE) + LN1 stats (DVE) as soon as x lands ----
            catx = cxpool.tile([P, NSUB, MO, P], BF16, tag="catx")
            for tt in range(NSUB):
                for kk in range(MO):
                    pe_transpose(catx[:, tt, kk, :],
                                 x_nat[:, tt, kk * P:(kk + 1) * P])

            agg = spool.tile([P, NSUB, 4], F32, tag="agg")
            rpack = spool.tile([P, 2 * NSUB], F32, tag="rpack")
            for tt in range(NSUB):
                bnx = spool.tile([P, 3, 6], F32, tag="bnx")
                for c in range(NCHUNK):
                    nc.vector.bn_stats(bnx[:, c, :],
                                       x_nat[:, tt, c * CH:(c + 1) * CH])
                nc.vector.bn_aggr(agg[:, tt, 0:2], bnx)
                nc.vector.tensor_scalar_add(rpack[:, 2 * tt:2 * tt + 1],
                                            agg[:, tt, 1:2], EPS)

            # ---- GEMM1 eager wave m0-4 opens its PSUM banks; the x-part
            # matmuls fire as soon as catx lands ----
            psA = [psapool.tile([P, TSUP], F32, tag="psA", name=f"psA{j}")
                   for j in range(5)]

            def rhs_k(kko):
                if kko < MO:
                    return catx[:, :, kko, :]
                return catf[:, :, kko - MO, :]

            def g1_matmul(ps, kko, mm):
                nc.tensor.matmul(
                    ps.rearrange("p (a b) -> p a b", a=NSUB),
                    lhsT=w1t_sb[:, kko, mm * P:(mm + 1) * P],
                    rhs=rhs_k(kko),
                    start=(kko == 0),
                    stop=(kko == KO1 - 1),
                )

            catf = cfpool.tile([P, NSUB, 2 * MO, P], BF16, tag="catf")

            for kko in range(0, MO):            # eager phase: x k-group
                for mm in range(5):
                    g1_matmul(psA[mm], kko, mm)

            # ---- layer sums: DVE chains per (half, subtile) ----
            def srcs_for(specs, tt):
                out = []
                for l0, j0, nj in specs:
                    for j in range(j0, j0 + nj):
                        out.append(blk[(l0, tt)][:, j, :])
                return out

            def chain_dv(name, specs, tailworks=()):
                d = apool.tile([P, NSUB, H], BF16, name=name, tag="acc")
                srcs = [srcs_for(specs, tt) for tt in range(NSUB)]
                tailworks = dict(tailworks)
                for j in range(1, len(srcs[0])):
                    for tt in range(NSUB):
                        s = srcs[tt]
                        if j == 1:
                            nc.vector.tensor_tensor(d[:, tt, :], s[0], s[1],
                                                    ALU.add)
                        else:
                            nc.vector.tensor_tensor(d[:, tt, :], d[:, tt, :],
                                                    s[j], ALU.add)
                    if j in tailworks:
                        tailworks[j]()
                return d

            d1 = chain_dv("d1", [(0, 0, 6), (6, 0, 6), (12, 0, 1)])

            # GEMM2 of the previous supertile, first subtile: fills the PE
            # gap while this supertile's d1 finishes; its fg stats follow
            # on DVE right after the d1 chain.
            fgs_p = []
            if prev is not None:
                fgs_p.append(emit_gemm2_tt(prev[1], 0))
                emit_bnf_tt(prev, fgs_p[0], 0)

            for tt in range(NSUB):
                for kk in range(MO):
                    pe_transpose(catf[:, tt, kk, :],
                                 d1[:, tt, kk * P:(kk + 1) * P])
            for kko in range(MO, 2 * MO):       # eager phase: f1 k-group
                for mm in range(5):
                    g1_matmul(psA[mm], kko, mm)

            if prev is not None:
                fgs_p.append(emit_gemm2_tt(prev[1], 1))

                def cb_stats():
                    emit_bnf_tt(prev, fgs_p[1], 1)
                    cb_stats.xns = emit_tail_norm(prev, fgs_p)

                def cb_combine():
                    emit_tail_combine(prev, cb_stats.xns)

                tails = ((2, cb_stats), (7, cb_combine))
            else:
                tails = ()

            d2 = chain_dv("d2", [(12, 1, 1), (14, 0, 6), (20, 0, 6)],
                          tailworks=tails)

            for tt in range(NSUB):
                for kk in range(MO):
                    pe_transpose(catf[:, tt, MO + kk, :],
                                 d2[:, tt, kk * P:(kk + 1) * P])
            for kko in range(2 * MO, KO1):      # eager phase: f2 k-group
                for mm in range(5):
                    g1_matmul(psA[mm], kko, mm)

            gT = gpool.tile([P, MO, TSUP], BF16, tag="gT")
            for mm in range(5):
                nc.scalar.activation(gT[:, mm, :], psA[mm], act,
                                     bias=b1_sb[:, mm:mm + 1])

            psB = [psapool.tile([P, TSUP], F32, tag="psA", name=f"psB{j}")
                   for j in range(4)]
            for kko in range(KO1):              # late sweep: m = 5..8
                for j in range(4):
                    g1_matmul(psB[j], kko, 5 + j)
            for j in range(4):
                nc.scalar.activation(gT[:, 5 + j, :], psB[j], act,
                                     bias=b1_sb[:, 5 + j:6 + j])

            prev = (x_nat, gT, agg, rpack, t0)

        # flush the last supertile: GEMM2 + LN tail
        fgs_p = [emit_gemm2_tt(prev[1], 0), emit_gemm2_tt(prev[1], 1)]
        emit_bnf_tt(prev, fgs_p[0], 0)
        emit_bnf_tt(prev, fgs_p[1], 1)
        xns = emit_tail_norm(prev, fgs_p)
        emit_tail_combine(prev, xns)


_NC_CACHE = {}


def _get_nc():
    if "nc" not in _NC_CACHE:
        _NC_CACHE["nc"] = _build_program()
    return _NC_CACHE["nc"]


def _host_prep(codes1, scales1, b1, codes2, scales2, b2,
               ln1_g, ln1_b, ln2_g, ln2_b, alpha):
    # W1 with 1/13 folded into the f1/f2 column blocks (mean -> sum)
    w1 = _dequant_nf4(codes1, scales1)
    # match reference rounding: dequant result is cast to bf16 first
    w1 = w1.astype(NP_BF16).astype(np.float32)
    w1[:, H:] *= np.float32(1.0 / 13.0)
    w1t = np.ascontiguousarray(w1.T).astype(NP_BF16)

    w2 = _dequant_nf4(codes2, scales2).astype(NP_BF16)
    w2t = np.ascontiguousarray(w2.astype(np.float32).T).astype(NP_BF16)
    # extended with the b2 row (k-tile 9 row 0) for the GEMM2 bias fold
    w2te = np.zeros(((KO2 + 1) * P, H), dtype=NP_BF16)
    w2te[:H] = w2t
    w2te[H] = b2.astype(NP_BF16)

    b1s = np.ascontiguousarray(
        b1.astype(np.float32).reshape(MO, P).T)  # [P, MO]

    a32 = alpha.astype(np.float32)
    w_gate = (1.0 / (1.0 + np.exp(-a32[0]))).astype(NP_BF16)
    one_minus = (NP_BF16(1.0) - w_gate)
    g1 = (one_minus.astype(np.float32) * ln1_g.astype(np.float32))
    g2 = (w_gate.astype(np.float32) * ln2_g.astype(np.float32))
    bc = (w_gate.astype(np.float32) * ln2_b.astype(np.float32)
          + one_minus.astype(np.float32) * ln1_b.astype(np.float32))
    g1b = np.ascontiguousarray(np.broadcast_to(g1.astype(NP_BF16), (P, H)))
    g2b = np.ascontiguousarray(np.broadcast_to(g2.astype(NP_BF16), (P, H)))
    bcb = np.ascontiguousarray(np.broadcast_to(bc.astype(NP_BF16), (P, H)))
    ident = np.eye(P, dtype=NP_BF16)
    return w1t, w2te, b1s, g1b, g2b, bcb, ident


def make_in_maps(image_features, hidden, codes1, scales1, b1, codes2, scales2,
                 b2, ln1_g, ln1_b, ln2_g, ln2_b, alpha):
    w1t, w2te, b1s, g1b, g2b, bcb, ident = _host_prep(
        codes1, scales1, b1, codes2, scales2, b2,
        ln1_g, ln1_b, ln2_g, ln2_b, alpha)
    B = image_features.shape[0]
    in_maps = []
    for c in range(B):
        in_maps.append({
            "x": np.ascontiguousarray(image_features[c]).astype(NP_BF16, copy=False),
            "hid": np.ascontiguousarray(hidden[:, c]).astype(NP_BF16, copy=False),
            "w1t": w1t, "w2t": w2te, "b1s": b1s,
            "g1b": g1b, "g2b": g2b, "bcb": bcb, "ident": ident,
        })
    return in_maps


def kernel(image_features, hidden, codes1, scales1, b1, codes2, scales2, b2,
           ln1_g, ln1_b, ln2_g, ln2_b, alpha, _trace=False):
    B, N, Hin = image_features.shape
    assert (B, N, Hin) == (8, NT, H), (B, N, Hin)
    nc = _get_nc()
    in_maps = make_in_maps(image_features, hidden, codes1, scales1, b1,
                           codes2, scales2, b2, ln1_g, ln1_b, ln2_g, ln2_b,
                           alpha)
    res = bass_utils.run_bass_kernel_spmd(
        nc, in_maps, core_ids=list(range(8)), trace=_trace)
    out = np.stack([res.results[c]["out"] for c in range(8)])
    if _trace:
        kernel._last_results = res
    return out.astype(image_features.dtype, copy=False)



# revision 19
# speedup vs baseline: 1.0352x; 1.0352x over previous
"""Trainium2 Bass kernel for nn_Connector_77738908057780 (dense_mlp).

Computation (see reference):
  x   = image_features                      [B, N, H]    bf16
  f1  = mean(hidden[0:13],  axis=0)         [B, N, H]
  f2  = mean(hidden[13:26], axis=0)         [B, N, H]
  cat = concat([x, f1, f2], -1)             [B, N, 3H]
  h   = gelu(cat @ W1.T + b1)               W1 = nf4_dequant(codes1, scales1) [H, 3H]
  fg  = h @ W2.T + b2                       W2 = nf4_dequant(codes2, scales2) [H, H]
  out = w * LN(fg) + (1-w) * LN(x),         w = sigmoid(alpha)

Sharding: data-parallel over batch B=8 -> one batch element per NeuronCore.

v3 design (vs the 247us v2):
  - supertile schedule [128, 256, 256, 128]: small first tile shortens the
    DMA-bound pipeline fill, small last tile shrinks the compute-only drain
    (v2 lost ~67us after the last DMA byte).
  - x arrives pre-transposed from host prep (xT [ko,128,729] contiguous per
    partition): the 18 PE transposes + ACT drains per supertile for the x
    k-group are gone, and GEMM1's x-phase fires directly off the DMA.
  - GEMM1 is k-eager over ALL 9 m-tiles: 9 accumulators packed 2-per-bank
    into 5 PSUM banks ([P,512] f32 tiles, two [P,256] accumulation regions
    each). No late m5-8 sweep on the tail.
  - hidden streamed as 7/6-layer blocks per chain half (d1 = layers 0..12,
    d2 = 13..25), summed by DVE chains at the 2x-mode TT ceiling.
  - skewed pipeline: supertile st's GEMM2/LN2/gate/store runs inside st+1's
    window (PE gap after the x-phase), so the tail is only the last 128
    tokens deep.
  - b2 folded into GEMM2 as a 10th k-tile; NF4 dequant is host-side.
"""

import os
import sys

import numpy as np
import ml_dtypes

for _p in ("/opt/trn_rl_repo", "/root/.axon_site/_ro/trn_rl_repo"):
    if os.path.isdir(_p) and _p not in sys.path:
        sys.path.insert(0, _p)

import concourse.bass as bass
import concourse.mybir as mybir
import concourse.tile as tile
from concourse import bacc
from concourse import bass_utils

BF16 = mybir.dt.bfloat16
F32 = mybir.dt.float32
AF = mybir.ActivationFunctionType
ALU = mybir.AluOpType

NP_BF16 = ml_dtypes.bfloat16

P = 128
H = 1152
H3 = 3456
NT = 729          # tokens per core (N); B=8 cores
L = 26
KO1 = H3 // P     # 27 k-tiles for GEMM1
KO2 = H // P      # 9 k-tiles for GEMM2 (+1 ones-tile for the b2 fold)
MO = H // P       # 9 output-feature tiles
EPS = 1e-5
NCHUNK = 3        # fg free-dim chunks of 384
CH = H // NCHUNK  # 384

# (t0, nsub): supertiles of nsub*128 tokens. Coverage 0..728 with tokens
# 601..639 computed twice (identical values stored twice) so every tile is
# a full 128-partition tile.
SUPERTILES = [(0, 1), (128, 2), (384, 2), (601, 1)]

# hidden layer blocks per chain: d1 = layers 0..12, d2 = 13..25
D1_BLOCKS = [(0, 7), (7, 6)]
D2_BLOCKS = [(13, 7), (20, 6)]

NF4_CODEBOOK = np.array([
    -1.0, -0.6961928009986877, -0.5250730514526367, -0.39491748809814453,
    -0.28444138169288635, -0.18477343022823334, -0.09105003625154495, 0.0,
    0.07958029955625534, 0.16093020141124725, 0.24611230194568634,
    0.33791524171829224, 0.4407098591327667, 0.5626170039176941,
    0.7229568362236023, 1.0], dtype=np.float32)

BLOCK = 64


def _dequant_nf4(codes, scales):
    """Match reference: codebook lookup * per-64-block absmax, cast bf16."""
    out_f, in_f = codes.shape
    w = NF4_CODEBOOK[codes].reshape(out_f, in_f // BLOCK, BLOCK)
    w = w * scales[:, :, None].astype(np.float32)
    return w.reshape(out_f, in_f)  # float32 (caller casts)


def _build_program(act=AF.Gelu, uniform_gate=False):
    nc = bacc.Bacc(
        "TRN2",
        target_bir_lowering=False,
        debug=False,
        num_devices=1,
    )
    x_d = nc.dram_tensor("x", (NT, H), BF16, kind="ExternalInput").ap()
    xt_d = nc.dram_tensor("xt", (MO * P, NT), BF16, kind="ExternalInput").ap()
    hid_d = nc.dram_tensor("hid", (L, NT, H), BF16, kind="ExternalInput").ap()
    w1t_d = nc.dram_tensor("w1t", (H3, H), BF16, kind="ExternalInput").ap()
    w2t_d = nc.dram_tensor("w2t", ((KO2 + 1) * P, H), BF16,
                           kind="ExternalInput").ap()
    b1s_d = nc.dram_tensor("b1s", (P, MO), F32, kind="ExternalInput").ap()
    g1b_d = nc.dram_tensor("g1b", (P, H), BF16, kind="ExternalInput").ap()
    g2b_d = nc.dram_tensor("g2b", (P, H), BF16, kind="ExternalInput").ap()
    bcb_d = nc.dram_tensor("bcb", (P, H), BF16, kind="ExternalInput").ap()
    id_d = nc.dram_tensor("ident", (P, P), BF16, kind="ExternalInput").ap()
    out_d = nc.dram_tensor("out", (NT, H), BF16, kind="ExternalOutput").ap()

    with tile.TileContext(nc) as tc:
        _program(nc, tc, x_d, xt_d, hid_d, w1t_d, w2t_d, b1s_d,
                 g1b_d, g2b_d, bcb_d, id_d, out_d, act, uniform_gate)

    nc.compile()
    return nc


def _program(nc, tc, x_d, xt_d, hid_d, w1t_d, w2t_d, b1s_d, g1b_d, g2b_d,
             bcb_d, id_d, out_d, act=AF.Gelu, uniform_gate=False):
    with (
        tc.tile_pool(name="consts", bufs=1) as cpool,
        tc.tile_pool(name="hp", bufs=3) as hpool,
        tc.tile_pool(name="acc", bufs=2) as apool,
        tc.tile_pool(name="catf", bufs=1) as cfpool,
        tc.tile_pool(name="gt", bufs=1) as gpool,
        tc.tile_pool(name="xn", bufs=2) as xpool,
        tc.tile_pool(name="fg", bufs=2) as fgpool,
        tc.tile_pool(name="outp", bufs=2) as opool,
        tc.tile_pool(name="stats", bufs=2) as spool,
        tc.tile_pool(name="tmp", bufs=1) as tpool,
        tc.tile_pool(name="psA", bufs=5, space="PSUM") as psapool,
        tc.tile_pool(name="ps2", bufs=1, space="PSUM") as ps2pool,
        tc.tile_pool(name="psT", bufs=2, space="PSUM") as pstpool,
    ):
        # ---- constants ----
        ones_sb = cpool.tile([P, P], BF16)
        nc.vector.memset(ones_sb[:, :], 1.0)
        cvec_sb = cpool.tile([P, 4], F32)
        id_sb = cpool.tile([P, P], BF16)
        nc.scalar.dma_start(id_sb, id_d)
        b1_sb = cpool.tile([P, MO], F32)
        g1b_sb = cpool.tile([P, H], BF16)
        g2b_sb = cpool.tile([P, H], BF16)
        bcb_sb = cpool.tile([P, H], BF16)
        w1t_sb = cpool.tile([P, KO1, H], BF16)
        w2t_sb = cpool.tile([P, KO2 + 1, H], BF16)
        xt_sb = cpool.tile([P, MO, NT], BF16)
        nc.scalar.dma_start(b1_sb, b1s_d)
        nc.scalar.dma_start(g1b_sb, g1b_d)
        nc.scalar.dma_start(g2b_sb, g2b_d)
        nc.scalar.dma_start(bcb_sb, bcb_d)
        if uniform_gate:
            # G1/G2 are uniform scalars: [c1, c2, c1, c2] columns for the
            # rsqrt-scale fold (combine collapses to xn1 + xn2)
            for s in range(4):
                src = g1b_sb if s % 2 == 0 else g2b_sb
                nc.vector.tensor_copy(cvec_sb[:, s:s + 1], src[:, 0:1])

        w1t_r = w1t_d.rearrange("(ko p) n -> p ko n", p=P)
        w2t_r = w2t_d.rearrange("(ko p) n -> p ko n", p=P)
        xt_r = xt_d.rearrange("(ko p) t -> p ko t", p=P)

        # xT leads the sync queue (small); the sync ring then carries ONLY
        # x_nat + hidden blocks + output stores. Weights ride the scalar
        # HWDGE ring (separate FIFO, SDMA engines round-robin between the
        # rings) so blocks stream from t=0 and weights arrive concurrently.
        nc.sync.dma_start(xt_sb, xt_r)
        for c0 in range(0, KO1, 9):
            nc.scalar.dma_start(w1t_sb[:, c0:c0 + 9, :],
                                w1t_r[:, c0:c0 + 9, :])
        nc.scalar.dma_start(w2t_sb, w2t_r)

        def pe_transpose(dst, src):
            """128x128 transpose on the TensorEngine (identity matmul,
            bf16 PSUM out), copied out by ACT."""
            psT = pstpool.tile([P, P], BF16, tag="psT", name="psT")
            nc.tensor.transpose(psT, src, id_sb)
            nc.scalar.activation(dst, psT, AF.Copy)

        def emit_gemm2_tt(gTp, tt):
            """GEMM2 for one 128-token subtile of the PREVIOUS supertile
            (+b2 via all-ones stationary x [b2;0..] k-tile), PSUM drained
            by ACT copies."""
            fg = fgpool.tile([P, H], BF16, tag="fg", name=f"fg{tt}")
            for nn in range(NCHUNK):
                ps2 = ps2pool.tile([P, 512], F32, tag="ps2", name="ps2")
                for kk in range(KO2):
                    nc.tensor.matmul(
                        ps2[:, 0:CH],
                        lhsT=gTp[:, kk, tt * P:(tt + 1) * P],
                        rhs=w2t_sb[:, kk, nn * CH:(nn + 1) * CH],
                        start=(kk == 0),
                        stop=False,
                    )
                nc.tensor.matmul(
                    ps2[:, 0:CH],
                    lhsT=ones_sb,
                    rhs=w2t_sb[:, KO2, nn * CH:(nn + 1) * CH],
                    start=False,
                    stop=True,
                )
                nc.scalar.activation(fg[:, nn * CH:(nn + 1) * CH],
                                     ps2[:, 0:CH], AF.Copy)
            return fg

        def emit_bnf_tt(pv, fg, tt):
            """LN2 stats for one prev subtile (DVE)."""
            p_agg, p_rpack = pv[2], pv[3]
            bnf = spool.tile([P, 3, 6], F32, tag="bnf")
            for c in range(NCHUNK):
                nc.vector.bn_stats(bnf[:, c, :], fg[:, c * CH:(c + 1) * CH])
            nc.vector.bn_aggr(p_agg[:, tt, 2:4], bnf)
            nc.vector.tensor_scalar_add(p_rpack[:, 2 * tt + 1:2 * tt + 2],
                                        p_agg[:, tt, 3:4], EPS)

        def emit_tail_norm(pv, fgs_p):
            """rsqrt + per-token normalizations xn = (v - mu) * rsqrt as
            Identity activations (scale=rsqrt, bias=-mu*rsqrt; both [P,1]
            APs)."""
            (p_x, p_gT, p_agg, p_rpack, p_t0, p_nsub) = pv
            ig = spool.tile([P, 4], F32, tag="ig")
            nc.vector.reciprocal(ig[:, 0:2 * p_nsub], p_rpack[:, 0:2 * p_nsub])
            nc.scalar.activation(ig[:, 0:2 * p_nsub], ig[:, 0:2 * p_nsub],
                                 AF.Sqrt)
            if uniform_gate:
                # fold the uniform gains into the normalize scales
                nc.vector.tensor_tensor(ig[:, 0:2 * p_nsub],
                                        ig[:, 0:2 * p_nsub],
                                        cvec_sb[:, 0:2 * p_nsub], ALU.mult)
            bv = spool.tile([P, 4], F32, tag="bv")
            xns = []
            for tt in range(p_nsub):
                for ln in range(2):
                    s = 2 * tt + ln
                    nc.vector.tensor_scalar(
                        bv[:, s:s + 1], p_agg[:, tt, 2 * ln:2 * ln + 1],
                        ig[:, s:s + 1], -1.0, ALU.mult, ALU.mult)
                xn1 = tpool.tile([P, H], BF16, tag="xn1")
                nc.scalar.activation(xn1, p_x[:, tt, :], AF.Identity,
                                     bias=bv[:, 2 * tt:2 * tt + 1],
                                     scale=ig[:, 2 * tt:2 * tt + 1])
                xn2 = tpool.tile([P, H], BF16, tag="xn2")
                nc.scalar.activation(xn2, fgs_p[tt], AF.Identity,
                                     bias=bv[:, 2 * tt + 1:2 * tt + 2],
                                     scale=ig[:, 2 * tt + 1:2 * tt + 2])
                xns.append((xn1, xn2))
            return xns

        def emit_tail_combine(pv, xns):
            """DVE all-bf16 2x TTs: out = xn1*G1 + xn2*G2 + Bc, then store.
            uniform_gate: gains folded into the normalize scales upstream,
            Bc asserted zero -> single add."""
            p_t0, p_nsub = pv[4], pv[5]
            for tt in range(p_nsub):
                xn1, xn2 = xns[tt]
                out_t = opool.tile([P, H], BF16, tag="outt")
                if uniform_gate:
                    nc.vector.tensor_tensor(out_t, xn1, xn2, ALU.add)
                else:
                    nc.vector.tensor_tensor(xn1, xn1, g1b_sb, ALU.mult)
                    nc.vector.tensor_tensor(xn2, xn2, g2b_sb, ALU.mult)
                    nc.vector.tensor_tensor(xn1, xn1, xn2, ALU.add)
                    nc.vector.tensor_tensor(out_t, xn1, bcb_sb, ALU.add)
                nc.sync.dma_start(
                    out_d[p_t0 + tt * P:p_t0 + (tt + 1) * P, :], out_t)

        prev = None

        for st_idx, (t0, nsub) in enumerate(SUPERTILES):
            ntok = nsub * P

            # ---- loads on the sync queue ----
            x_nat = xpool.tile([P, 2, H], BF16, tag="xnat")
            nc.sync.dma_start(
                x_nat[:, 0:nsub, :],
                x_d[t0:t0 + ntok, :].rearrange("(s p) f -> p s f", p=P),
            )

            blk = {}
            for l0, nl in D1_BLOCKS + D2_BLOCKS:
                for tt in range(nsub):
                    bt = hpool.tile([P, 7, H], BF16, name=f"b{l0}t{tt}",
                                    tag="hp")
                    nc.sync.dma_start(
                        bt[:, 0:nl, :],
                        hid_d[l0:l0 + nl,
                              t0 + tt * P:t0 + (tt + 1) * P, :].rearrange(
                            "l p f -> p l f"))
                    blk[(l0, tt)] = bt

            # ---- LN1 stats (DVE) as soon as x lands ----
            agg = spool.tile([P, 2, 4], F32, tag="agg")
            rpack = spool.tile([P, 4], F32, tag="rpack")
            for tt in range(nsub):
                bnx = spool.tile([P, 3, 6], F32, tag="bnx")
                for c in range(NCHUNK):
                    nc.vector.bn_stats(bnx[:, c, :],
                                       x_nat[:, tt, c * CH:(c + 1) * CH])
                nc.vector.bn_aggr(agg[:, tt, 0:2], bnx)
                nc.vector.tensor_scalar_add(rpack[:, 2 * tt:2 * tt + 1],
                                            agg[:, tt, 1:2], EPS)

            # ---- GEMM1: 9 accumulators in 5 PSUM banks, k-eager ----
            psA = [psapool.tile([P, 512], F32, tag="psA", name=f"psA{j}")
                   for j in range(5)]
            accs = []
            for mm in range(MO):
                half = (mm % 2) * 256
                accs.append(psA[mm // 2][:, half:half + ntok])

            def g1_matmul(kko, mm, rhs, shaped=False):
                """rhs: flat [P, ntok] (shaped=False) or [P, nsub, 128].

                PSUM packs two accumulation groups per bank (mm=2j, 2j+1).
                start=True clears the has_written bits for the WHOLE bank,
                so only the first matmul in each bank (even mm at kko=0)
                may set it; the odd group's first matmul relies on its bits
                being freshly cleared (flags=0 on a clear bit = overwrite)."""
                dst = accs[mm]
                if shaped and nsub == 2:
                    dst = dst.rearrange("p (a b) -> p a b", a=nsub)
                nc.tensor.matmul(
                    dst,
                    lhsT=w1t_sb[:, kko, mm * P:(mm + 1) * P],
                    rhs=rhs,
                    start=(kko == 0 and mm % 2 == 0),
                    stop=(kko == KO1 - 1),
                    skip_group_check=True,
                )

            for kko in range(0, MO):            # eager phase: x k-group
                for mm in range(MO):
                    g1_matmul(kko, mm, xt_sb[:, kko, t0:t0 + ntok])

            # ---- layer sums: DVE chains per (half, subtile) ----
            def chain_dv(name, specs):
                d = apool.tile([P, 2, H], BF16, name=name, tag="acc")
                srcs = [[blk[(l0, tt)][:, j, :] for l0, nl in specs
                         for j in range(nl)] for tt in range(nsub)]
                for j in range(1, len(srcs[0])):
                    for tt in range(nsub):
                        s = srcs[tt]
                        if j == 1:
                            nc.vector.tensor_tensor(d[:, tt, :], s[0], s[1],
                                                    ALU.add)
                        else:
                            nc.vector.tensor_tensor(d[:, tt, :], d[:, tt, :],
                                                    s[j], ALU.add)
                return d

            catf = cfpool.tile([P, 2, 2 * MO, P], BF16, tag="catf")

            def f_slab_rhs(ci):
                if nsub == 2:
                    return catf[:, 0:2, ci, :]
                return catf[:, 0, ci, :]

            def f_phase(d, base_ko):
                """Interleave per-slab transposes with the previous slab's
                9 eager matmuls so PE never waits on the ACT psT drain."""
                for kk in range(MO):
                    for tt in range(nsub):
                        pe_transpose(catf[:, tt, base_ko - MO + kk, :],
                                     d[:, tt, kk * P:(kk + 1) * P])
                    if kk > 0:
                        for mm in range(MO):
                            g1_matmul(base_ko + kk - 1, mm,
                                      f_slab_rhs(base_ko - MO + kk - 1),
                                      shaped=True)
                for mm in range(MO):
                    g1_matmul(base_ko + MO - 1, mm, f_slab_rhs(base_ko - 1),
                              shaped=True)

            d1 = chain_dv("d1", D1_BLOCKS)

            # prev supertile's GEMM2 fills the PE gap while d1 finishes.
            # Its DVE-side stats are emitted AFTER the d2 chain so a late
            # GEMM2 can never head-of-line-block the chains (which gate
            # hidden-buffer recycling and hence the DMA).
            fgs_p = []
            if prev is not None:
                for tt in range(prev[5]):
                    fgs_p.append(emit_gemm2_tt(prev[1], tt))

            f_phase(d1, MO)

            d2 = chain_dv("d2", D2_BLOCKS)

            if prev is not None:
                for tt in range(prev[5]):
                    emit_bnf_tt(prev, fgs_p[tt], tt)
                xns = emit_tail_norm(prev, fgs_p)
                emit_tail_combine(prev, xns)

            f_phase(d2, 2 * MO)

            gT = gpool.tile([P, MO, 256], BF16, tag="gT")
            for mm in range(MO):
                nc.scalar.activation(gT[:, mm, 0:ntok], accs[mm], act,
                                     bias=b1_sb[:, mm:mm + 1])

            prev = (x_nat, gT, agg, rpack, t0, nsub)

        # flush the last supertile: GEMM2 + LN tail
        fgs_p = []
        for tt in range(prev[5]):
            fgs_p.append(emit_gemm2_tt(prev[1], tt))
            emit_bnf_tt(prev, fgs_p[tt], tt)
        xns = emit_tail_norm(prev, fgs_p)
        emit_tail_combine(prev, xns)


_NC_CACHE = {}


def _get_nc(uniform_gate=False):
    key = ("nc", uniform_gate)
    if key not in _NC_CACHE:
        _NC_CACHE[key] = _build_program(uniform_gate=uniform_gate)
    return _NC_CACHE[key]


def _detect_uniform(ln1_g, ln1_b, ln2_g, ln2_b):
    """True when the LN gains are uniform and biases zero: the gate/gain
    factors collapse to scalars folded into the normalize scales."""
    return bool(
        np.all(ln1_g == np.asarray(ln1_g).flat[0])
        and np.all(ln2_g == np.asarray(ln2_g).flat[0])
        and np.all(np.asarray(ln1_b) == 0)
        and np.all(np.asarray(ln2_b) == 0)
    )


def _host_prep(codes1, scales1, b1, codes2, scales2, b2,
               ln1_g, ln1_b, ln2_g, ln2_b, alpha):
    # W1 with 1/13 folded into the f1/f2 column blocks (mean -> sum)
    w1 = _dequant_nf4(codes1, scales1)
    # match reference rounding: dequant result is cast to bf16 first
    w1 = w1.astype(NP_BF16).astype(np.float32)
    w1[:, H:] *= np.float32(1.0 / 13.0)
    w1t = np.ascontiguousarray(w1.T).astype(NP_BF16)

    w2 = _dequant_nf4(codes2, scales2).astype(NP_BF16)
    w2t = np.ascontiguousarray(w2.astype(np.float32).T).astype(NP_BF16)
    # extended with the b2 row (k-tile 9 row 0) for the GEMM2 bias fold
    w2te = np.zeros(((KO2 + 1) * P, H), dtype=NP_BF16)
    w2te[:H] = w2t
    w2te[H] = b2.astype(NP_BF16)

    b1s = np.ascontiguousarray(
        b1.astype(np.float32).reshape(MO, P).T)  # [P, MO]

    a32 = alpha.astype(np.float32)
    w_gate = (1.0 / (1.0 + np.exp(-a32[0]))).astype(NP_BF16)
    one_minus = (NP_BF16(1.0) - w_gate)
    g1 = (one_minus.astype(np.float32) * ln1_g.astype(np.float32))
    g2 = (w_gate.astype(np.float32) * ln2_g.astype(np.float32))
    bc = (w_gate.astype(np.float32) * ln2_b.astype(np.float32)
          + one_minus.astype(np.float32) * ln1_b.astype(np.float32))
    g1b = np.ascontiguousarray(np.broadcast_to(g1.astype(NP_BF16), (P, H)))
    g2b = np.ascontiguousarray(np.broadcast_to(g2.astype(NP_BF16), (P, H)))
    bcb = np.ascontiguousarray(np.broadcast_to(bc.astype(NP_BF16), (P, H)))
    ident = np.eye(P, dtype=NP_BF16)
    return w1t, w2te, b1s, g1b, g2b, bcb, ident


def make_in_maps(image_features, hidden, codes1, scales1, b1, codes2, scales2,
                 b2, ln1_g, ln1_b, ln2_g, ln2_b, alpha):
    w1t, w2te, b1s, g1b, g2b, bcb, ident = _host_prep(
        codes1, scales1, b1, codes2, scales2, b2,
        ln1_g, ln1_b, ln2_g, ln2_b, alpha)
    B = image_features.shape[0]
    in_maps = []
    for c in range(B):
        xc = np.ascontiguousarray(image_features[c]).astype(NP_BF16,
                                                            copy=False)
        in_maps.append({
            "x": xc,
            "xt": np.ascontiguousarray(xc.T),
            "hid": np.ascontiguousarray(hidden[:, c]).astype(NP_BF16,
                                                             copy=False),
            "w1t": w1t, "w2t": w2te, "b1s": b1s,
            "g1b": g1b, "g2b": g2b, "bcb": bcb, "ident": ident,
        })
    return in_maps


def kernel(image_features, hidden, codes1, scales1, b1, codes2, scales2, b2,
           ln1_g, ln1_b, ln2_g, ln2_b, alpha, _trace=False):
    B, N, Hin = image_features.shape
    assert (B, N, Hin) == (8, NT, H), (B, N, Hin)
    nc = _get_nc(_detect_uniform(ln1_g, ln1_b, ln2_g, ln2_b))
    in_maps = make_in_maps(image_features, hidden, codes1, scales1, b1,
                           codes2, scales2, b2, ln1_g, ln1_b, ln2_g, ln2_b,
                           alpha)
    res = bass_utils.run_bass_kernel_spmd(
        nc, in_maps, core_ids=list(range(8)), trace=_trace)
    out = np.stack([res.results[c]["out"] for c in range(8)])
    if _trace:
        kernel._last_results = res
    return out.astype(image_features.dtype, copy=False)
